# revision 7
# baseline (speedup 1.0000x reference)
"""Trainium2 Bass kernel for nn_AttentionLayerDecoder (sparse segment attention).

Math (reference, edge_index unused):
  qk[h,b,:]   = Wk[h] @ (context[b] @ Wq[h]) / 8          (tiny, host)
  u[h,n]      = x[n,:] . qk[h,batch[n],:]
  e[h,n]      = exp(u[h,n])                                (u ~ N(0,1))
  xe[h,b,:]   = sum_{n in b} e[h,n] * x[n,:]               (device)
  S[h,b]      = sum_{n in b} e[h,n]                        (device)
  out         = (qc*query + (xe @ Wv)/S) @ Wf, summed over heads  (tiny, host)

Device design (CoreSim cost-model driven):
  - A matmul is charged only for its OUTPUT free size (weight load is free),
    so both big contractions are expressed with x as the *stationary* operand:
      u:  out[node,h]  = lhsT(x^T [f,node]) ^T @ qk[f,h]     -> 8 rows/tile
      xe: out[f,h]    += lhsT(x   [node,f]) ^T @ e[node,h]   -> 8 rows/tile
  - A DMA's transfer bytes are charged to the issuing queue; SP, Activation
    (HWDGE) and Pool (SWDGE) queues run in parallel, so x is shipped in fp8
    (rel err 3e-3 << 2e-2 budget) and chunks are spread greedily.
  - x is needed in BOTH layouts. "Natural" graphs get a second fp8 DMA copy
    (node-major). "Packed" graphs instead build the node-major copy on-chip:
    an int16 PE transpose of the f-major slab moves TWO fp8 nodes per PE row
    (columns pair-packed as int16), and the xe matmul consumes the result
    through a stride-2 fp8 view with even/odd node parities done as two
    accumulating matmuls. This trades cheap PE/DVE time for DMA queue time.
  - exp runs on ACT in batches; S via a ones-matmul; only [128,8]-per-graph
    results are ever evacuated from PSUM.
"""

import sys

if "/opt/trn_rl_repo" not in sys.path:
    sys.path.insert(0, "/opt/trn_rl_repo")

from contextlib import ExitStack

import ml_dtypes
import numpy as np

import concourse.bass as bass
import concourse.tile as tile
from concourse import bacc, masks, mybir
from concourse.bass_utils import run_bass_kernel_spmd

N_CORES = 8
H = 8          # heads
F = 128        # node feature dim
B = 128        # graphs
GPC = B // N_CORES  # graphs per core

K_PACK = 10            # graphs whose node-major copy is built by PE transpose
NAT = GPC - K_PACK     # graphs that get a second (node-major) DMA copy

FP8 = ml_dtypes.float8_e4m3

_CACHE = {}


def _build(G):
    T = G // 128
    GP = ((G + 255) // 256) * 256   # packed graphs padded to 256-multiples
    QP = GP // 256                  # 256-node groups per packed graph
    TH = T * H                      # e columns per natural graph
    PH = QP * 2 * H                 # e columns per packed graph

    nc = bacc.Bacc(None, target_bir_lowering=False)
    fp8 = mybir.dt.float8e4
    fp16 = mybir.dt.float16
    f32 = mybir.dt.float32
    AFT = mybir.ActivationFunctionType

    XTB = nc.dram_tensor("XTB", [max(NAT, 1), F, G], fp8, kind="ExternalInput")
    XN = nc.dram_tensor("XN", [max(NAT, 1), 128, T * F], fp8, kind="ExternalInput")
    XTA = nc.dram_tensor("XTA", [max(K_PACK, 1), F, GP], fp8, kind="ExternalInput")
    QKT = nc.dram_tensor("QKT", [F, GPC * H], fp16, kind="ExternalInput")
    OUT = nc.dram_tensor("OUT", [128, GPC * H + GPC], f32, kind="ExternalOutput")

    # e-column offsets: natural graphs (width TH) then packed (width PH)
    e_off = []
    off = 0
    for g in range(GPC):
        e_off.append(off)
        off += TH if g < NAT else PH
    e_tot = off
    # exp batches: consecutive graphs, PSUM bank limit 512 f32 columns
    groups = []
    cur, w = [], 0
    for g in range(GPC):
        gw = TH if g < NAT else PH
        if w + gw > 512:
            groups.append(cur)
            cur, w = [], 0
        cur.append(g)
        w += gw
    groups.append(cur)

    with tile.TileContext(nc) as tc, ExitStack() as ctx:
        const = ctx.enter_context(tc.tile_pool(name="const", bufs=1))
        xpool = ctx.enter_context(tc.tile_pool(name="x", bufs=1))
        epool = ctx.enter_context(tc.tile_pool(name="e", bufs=1))
        outp = ctx.enter_context(tc.tile_pool(name="outp", bufs=1))
        ps_u = ctx.enter_context(
            tc.tile_pool(name="ps_u", bufs=2, space=bass.MemorySpace.PSUM)
        )
        ps_t = ctx.enter_context(
            tc.tile_pool(name="ps_t", bufs=3, space=bass.MemorySpace.PSUM)
        )
        ps_o = ctx.enter_context(
            tc.tile_pool(name="ps_o", bufs=1, space=bass.MemorySpace.PSUM)
        )

        qkt = const.tile([F, GPC * H], fp16)
        ones = const.tile([128, 1], fp16)
        warm = const.tile([1, 1], f32)
        warm2 = const.tile([1, 1], fp16)
        ident = const.tile([128, 128], fp16)
        out_sb = outp.tile([128, GPC * H + GPC], f32)
        nc.gpsimd.memset(ones[:], 1.0)
        nc.gpsimd.memset(warm[:], 0.0)
        nc.gpsimd.memset(out_sb[:, GPC * H :], 0.0)
        masks.make_identity(nc, ident[:])
        # Warm the ACT exp table at t~0 (1283ns table load hides under DMA).
        nc.scalar.activation(warm2[:], warm[:], AFT.Exp)

        xtb_all = xpool.tile([128, max(NAT, 1) * G], fp8)
        xn_all = xpool.tile([128, max(NAT, 1) * T * F], fp8)
        xta_all = xpool.tile([128, max(K_PACK, 1) * GP], fp8)
        xnp = [xpool.tile([128, QP * 128], fp16, name=f"xnp{j}") for j in range(K_PACK)]
        e_all = epool.tile([128, e_tot], fp16)

        # ---- DMA schedule: greedy by projected queue completion ----
        CH = 2
        qload = {"sp": 1000.0, "pool": 0.0, "act": 1283.0 + 450.0 * len(groups)}
        qeng = {"sp": nc.sync, "pool": nc.gpsimd, "act": nc.scalar}

        nc.sync.dma_start(qkt[:], QKT[:])

        def issue(dst, src, cost):
            q = min(qload, key=lambda k: qload[k])
            qload[q] += cost
            qeng[q].dma_start(dst, src)

        for c in range(0, NAT, CH):
            n = min(CH, NAT - c)
            issue(
                xtb_all[:, c * G : (c + n) * G].rearrange("p (g n) -> p g n", g=n),
                XTB[c : c + n].rearrange("g f n -> f g n"),
                n * G * 0.3855,
            )
        for c in range(0, K_PACK, CH):
            n = min(CH, K_PACK - c)
            issue(
                xta_all[:, c * GP : (c + n) * GP].rearrange("p (g n) -> p g n", g=n),
                XTA[c : c + n].rearrange("g f n -> f g n"),
                n * GP * 0.3855,
            )
        for c in range(0, NAT, CH):
            n = min(CH, NAT - c)
            issue(
                xn_all[:, c * T * F : (c + n) * T * F].rearrange(
                    "p (g n) -> p g n", g=n
                ),
                XN[c : c + n].rearrange("g f n -> f g n"),
                n * T * F * 0.3855,
            )

        # ---- u matmuls + transposes (packed), exp per batch ----
        def u_graph(g, u_ps, base):
            if g < NAT:
                for t in range(T):
                    nc.tensor.matmul(
                        u_ps[:, base + t * H : base + (t + 1) * H],
                        xtb_all[:, g * G + t * 128 : g * G + (t + 1) * 128],
                        qkt[:, g * H : (g + 1) * H],
                        start=True,
                        stop=True,
                    )
            else:
                j = g - NAT
                sl = xta_all[:, j * GP : (j + 1) * GP]
                for q in range(QP):
                    v = sl[:, q * 256 : (q + 1) * 256].rearrange(
                        "f (n two) -> f two n", two=2
                    )
                    for par in range(2):
                        nc.tensor.matmul(
                            u_ps[:, base + (q * 2 + par) * H : base + (q * 2 + par + 1) * H],
                            v[:, par, :],
                            qkt[:, g * H : (g + 1) * H],
                            start=True,
                            stop=True,
                        )

        for grp in groups:
            w = sum(TH if g < NAT else PH for g in grp)
            u_ps = ps_u.tile([128, w], f32, tag="u", name=f"u{grp[0]}")
            base = 0
            for g in grp:
                u_graph(g, u_ps, base)
                base += TH if g < NAT else PH
            # transposes for packed graphs in this batch (PE, data = XTA)
            for g in grp:
                if g >= NAT:
                    j = g - NAT
                    tp = ps_t.tile([128, QP * 128], fp16, tag="tp", name=f"tp{j}")
                    sl = xta_all[:, j * GP : (j + 1) * GP].bitcast(fp16)
                    for q in range(QP):
                        nc.tensor.transpose(
                            tp[:, q * 128 : (q + 1) * 128],
                            sl[:, q * 128 : (q + 1) * 128],
                            ident[:],
                        )
                    nc.vector.tensor_copy(xnp[j][:], tp[:])
            nc.scalar.activation(
                e_all[:, e_off[grp[0]] : e_off[grp[0]] + w], u_ps[:], AFT.Exp
            )

        # ---- S matmuls (only need e), evacuated early ----
        s_a = ps_o.tile([TH, max(NAT, 1)], f32)
        s_b = ps_o.tile([PH, max(K_PACK, 1)], f32)
        for g in range(GPC):
            if g < NAT:
                nc.tensor.matmul(
                    s_a[:, g : g + 1],
                    e_all[:, e_off[g] : e_off[g] + TH],
                    ones[:],
                    start=True,
                    stop=True,
                )
            else:
                j = g - NAT
                nc.tensor.matmul(
                    s_b[:, j : j + 1],
                    e_all[:, e_off[g] : e_off[g] + PH],
                    ones[:],
                    start=True,
                    stop=True,
                )

        # ---- xe matmuls: packed first (inputs ready early), natural last ----
        xe_ps = ps_o.tile([128, GPC * H], f32)
        for g in range(NAT, GPC):
            j = g - NAT
            for q in range(QP):
                v = xnp[j][:, q * 128 : (q + 1) * 128].bitcast(fp8).rearrange(
                    "p (f two) -> p two f", two=2
                )
                for par in range(2):
                    nc.tensor.matmul(
                        xe_ps[:, g * H : (g + 1) * H],
                        v[:, par, :],
                        e_all[:, e_off[g] + (q * 2 + par) * H : e_off[g] + (q * 2 + par + 1) * H],
                        start=(q == 0 and par == 0),
                        stop=(q == QP - 1 and par == 1),
                    )
        if NAT > 0:
            nc.vector.tensor_copy(out_sb[0:TH, GPC * H : GPC * H + NAT], s_a[:])
        if K_PACK > 0:
            nc.vector.tensor_copy(out_sb[0:PH, GPC * H + NAT :], s_b[:])
        for g in range(NAT):
            for t in range(T):
                nc.tensor.matmul(
                    xe_ps[:, g * H : (g + 1) * H],
                    xn_all[:, g * T * F + t * F : g * T * F + (t + 1) * F],
                    e_all[:, e_off[g] + t * H : e_off[g] + (t + 1) * H],
                    start=(t == 0),
                    stop=(t == T - 1),
                )
        nc.vector.tensor_copy(out_sb[:, 0 : GPC * H], xe_ps[:])
        nc.sync.dma_start(OUT[:], out_sb[:])

    nc.compile()
    return nc


def _get(G):
    if G not in _CACHE:
        _CACHE[G] = _build(G)
    return _CACHE[G]


def _prepare(x, batch, context, Wq, Wk):
    """Host-side shard prep. Returns (in_maps, G, query, n_pad)."""
    counts = np.bincount(batch, minlength=B).astype(np.int64)
    G = int(np.ceil(max(int(counts.max()), 1) / 128.0) * 128)
    T = G // 128
    GP = ((G + 255) // 256) * 256
    starts = np.zeros(B + 1, np.int64)
    np.cumsum(counts, out=starts[1:])

    query = np.einsum("bc,hcv->hbv", context, Wq).astype(np.float32)  # [H,B,Dv]
    qk = np.einsum("hbv,hev->hbe", query, Wk).astype(np.float32)      # [H,B,F]
    qk8 = (qk / 8.0).astype(np.float16)

    x8 = x.astype(FP8)

    n_pad = np.zeros(B, np.float32)
    in_maps = []
    for c in range(N_CORES):
        XTBc = np.zeros((max(NAT, 1), F, G), FP8)
        XNc = np.zeros((max(NAT, 1), 128, T * F), FP8)
        XTAc = np.zeros((max(K_PACK, 1), F, GP), FP8)
        QKTc = np.zeros((F, GPC * H), np.float16)
        for gi in range(GPC):
            b = c * GPC + gi
            n0, n1 = int(starts[b]), int(starts[b + 1])
            cnt = n1 - n0
            if gi < NAT:
                XTBc[gi, :, 0:cnt] = x8[n0:n1].T
                buf = np.zeros((T * 128, F), FP8)
                buf[0:cnt] = x8[n0:n1]
                XNc[gi] = (
                    buf.reshape(T, 128, F).transpose(1, 0, 2).reshape(128, T * F)
                )
                n_pad[b] = G - cnt
            else:
                XTAc[gi - NAT, :, 0:cnt] = x8[n0:n1].T
                n_pad[b] = GP - cnt
            QKTc[:, gi * H : (gi + 1) * H] = qk8[:, b, :].T
        in_maps.append({"XTB": XTBc, "XN": XNc, "XTA": XTAc, "QKT": QKTc})
    return in_maps, G, query, n_pad


def kernel(**inputs):
    x = np.asarray(inputs["x"], np.float32)
    batch = np.asarray(inputs["batch"]).astype(np.int64)
    context = np.asarray(inputs["context"], np.float32)
    Wq = np.asarray(inputs["Wq"], np.float32)
    Wk = np.asarray(inputs["Wk"], np.float32)
    Wv = np.asarray(inputs["Wv"], np.float32)
    qc = float(np.asarray(inputs["query_coef"]).reshape(-1)[0])
    Wf = np.asarray(inputs["Wf"], np.float32)

    in_maps, G, query, n_pad = _prepare(x, batch, context, Wq, Wk)
    T = G // 128
    GP = ((G + 255) // 256) * 256
    QP = GP // 256

    nc = _get(G)
    res = run_bass_kernel_spmd(nc, in_maps, core_ids=list(range(N_CORES)))

    XE = np.zeros((H, B, F), np.float32)
    S = np.zeros((H, B), np.float32)
    for c in range(N_CORES):
        out = res.results[c]["OUT"]                      # [128, GPC*H + GPC]
        xeT = out[:, 0 : GPC * H].reshape(F, GPC, H)     # [f, g, h]
        XE[:, c * GPC : (c + 1) * GPC, :] = xeT.transpose(2, 1, 0)
        s = out[:, GPC * H :]
        if NAT > 0:
            sa = s[0 : T * H, 0:NAT].reshape(T, H, NAT)
            S[:, c * GPC : c * GPC + NAT] = sa.sum(axis=0)
        if K_PACK > 0:
            sb = s[0 : QP * 2 * H, NAT:GPC].reshape(QP * 2, H, K_PACK)
            S[:, c * GPC + NAT : (c + 1) * GPC] = sb.sum(axis=0)

    S = S - n_pad[None, :]  # pad rows contributed exp(0)*1 each to S
    Y = np.einsum("hbe,hev->hbv", XE, Wv.astype(np.float32))
    agg = Y / (S[..., None] + 1e-16)
    hbv = qc * query + agg
    out = np.einsum("hbv,ve->be", hbv, Wf)
    return out.astype(np.float32)


# revision 8
# speedup vs baseline: 1.1781x; 1.1781x over previous
"""Trainium2 Bass kernel for nn_AttentionLayerDecoder (sparse segment attention).

Math (reference, edge_index unused):
  qk[h,b,:]   = Wk[h] @ (context[b] @ Wq[h]) / 8          (tiny, host)
  u[h,n]      = x[n,:] . qk[h,batch[n],:]
  e[h,n]      = exp(u[h,n])                                (u ~ N(0,1))
  xe[h,b,:]   = sum_{n in b} e[h,n] * x[n,:]               (device)
  S[h,b]      = sum_{n in b} e[h,n]                        (device)
  out         = (qc*query + (xe @ Wv)/S) @ Wf, summed over heads  (tiny, host)

Device design (CoreSim cost-model driven):
  - A matmul is charged only for its OUTPUT free size (weight load is free),
    so both big contractions keep x as the *stationary* operand:
      u:  out[node,h]  = lhsT(x^T [f,node]) ^T @ qk[f,h]     -> 8 rows/tile
      xe: out[f,h]    += lhsT(x   [node,f]) ^T @ e[node,h]   -> 8 rows/tile
  - DMA transfer bytes are charged to the issuing queue; SP / Activation
    (HWDGE) / Pool (SWDGE) run in parallel. x ships in fp8 (rel err 3e-3 <<
    2e-2 budget) from one flat f-major tensor XALL = [qkt | packed | natural],
    sliced into ~2-graph chunks spread greedily over the three queues.
  - x is needed in BOTH layouts. "Natural" graphs get a second fp8 DMA copy
    (node-major, tensor XN). "Packed" graphs (processed first) instead build
    it on-chip: an fp16-viewed PE transpose moves TWO fp8 nodes per PE row,
    DVE evacuates two graphs per copy, and the xe matmul reads a stride-2
    fp8 view with even/odd node parities as two accumulating matmuls.
  - exp on ACT in two ~[128,480] batches; S via ones-matmuls; the xe PSUM is
    split so the last graph's matmuls never wait on the bulk evacuation.
"""

import sys

if "/opt/trn_rl_repo" not in sys.path:
    sys.path.insert(0, "/opt/trn_rl_repo")

from contextlib import ExitStack

import ml_dtypes
import numpy as np

import concourse.bass as bass
import concourse.tile as tile
from concourse import bacc, masks, mybir
from concourse.bass_utils import run_bass_kernel_spmd

N_CORES = 8
H = 8          # heads
F = 128        # node feature dim
B = 128        # graphs
GPC = B // N_CORES  # graphs per core

K_PACK = 6             # graphs whose node-major copy is built by PE transpose
NAT = GPC - K_PACK     # graphs that get a second (node-major) DMA copy

FP8 = ml_dtypes.float8_e4m3

_CACHE = {}


def _build(G):
    T = G // 128
    GP = ((G + 255) // 256) * 256   # packed graphs padded to 256-multiples
    QP = GP // 256                  # 256-node groups per packed graph
    TH = T * H                      # e columns per natural graph
    PH = QP * 2 * H                 # e columns per packed graph

    nc = bacc.Bacc(None, target_bir_lowering=False)
    fp8 = mybir.dt.float8e4
    fp16 = mybir.dt.float16
    f32 = mybir.dt.float32
    AFT = mybir.ActivationFunctionType

    W = 256 + K_PACK * GP + NAT * G   # qkt | packed slabs | natural slabs
    XALL = nc.dram_tensor("XALL", [F, W], fp8, kind="ExternalInput")
    XN = nc.dram_tensor("XN", [128, max(NAT, 1) * T * F], fp8, kind="ExternalInput")
    OUT = nc.dram_tensor("OUT", [128, GPC * H + GPC], f32, kind="ExternalOutput")

    def xoff(g):          # column offset of graph g's f-major slab in XALL
        if g < K_PACK:
            return 256 + g * GP
        return 256 + K_PACK * GP + (g - K_PACK) * G

    # e-column offsets: packed graphs (width PH) then natural (width TH)
    e_off, off = [], 0
    for g in range(GPC):
        e_off.append(off)
        off += PH if g < K_PACK else TH
    e_tot = off
    # exp batches (consecutive graphs, PSUM bank limit 512 f32 columns)
    groups, cur, w = [], [], 0
    for g in range(GPC):
        gw = PH if g < K_PACK else TH
        if w + gw > 512:
            groups.append(cur)
            cur, w = [], 0
        cur.append(g)
        w += gw
    groups.append(cur)

    with tile.TileContext(nc) as tc, ExitStack() as ctx:
        const = ctx.enter_context(tc.tile_pool(name="const", bufs=1))
        xpool = ctx.enter_context(tc.tile_pool(name="x", bufs=1))
        epool = ctx.enter_context(tc.tile_pool(name="e", bufs=1))
        outp = ctx.enter_context(tc.tile_pool(name="outp", bufs=1))
        ps_u = ctx.enter_context(
            tc.tile_pool(name="ps_u", bufs=2, space=bass.MemorySpace.PSUM)
        )
        ps_t = ctx.enter_context(
            tc.tile_pool(name="ps_t", bufs=2, space=bass.MemorySpace.PSUM)
        )
        ps_o = ctx.enter_context(
            tc.tile_pool(name="ps_o", bufs=1, space=bass.MemorySpace.PSUM)
        )

        ones = const.tile([128, 1], fp16)
        warm = const.tile([1, 1], f32)
        warm2 = const.tile([1, 1], fp16)
        ident = const.tile([128, 128], fp16)
        out_sb = outp.tile([128, GPC * H + GPC], f32)
        nc.gpsimd.memset(ones[:], 1.0)
        nc.gpsimd.memset(warm[:], 0.0)
        nc.gpsimd.memset(out_sb[:, GPC * H :], 0.0)
        masks.make_identity(nc, ident[:])
        # Warm the ACT exp table at t~0 (1283ns table load hides under DMA).
        nc.scalar.activation(warm2[:], warm[:], AFT.Exp)

        xall = xpool.tile([128, W], fp8)
        xn_all = xpool.tile([128, max(NAT, 1) * T * F], fp8)
        # one SBUF tile per packed PAIR: [g, g+1] transposed, fp16-packed
        xnp = [
            xpool.tile([128, 2 * QP * 128], fp16, name=f"xnp{j}")
            for j in range((K_PACK + 1) // 2)
        ]
        e_all = epool.tile([128, e_tot], fp16)
        qkt = xall[:, 0:256].bitcast(fp16)   # [128, GPC*H]

        # ---- DMA schedule: greedy by projected queue completion ----
        qload = {"sp": 500.0, "pool": 0.0, "act": 1283.0 + 1200.0}
        qeng = {"sp": nc.sync, "pool": nc.gpsimd, "act": nc.scalar}

        def issue_cols(a, b):
            q = min(qload, key=lambda k: qload[k])
            qload[q] += (b - a) * 0.3855
            qeng[q].dma_start(xall[:, a:b], XALL[:, a:b])

        # first chunk carries qkt + packed pair 0; then packed pairs, natural
        # pairs, and finally the XN (node-major) chunks that feed the tail.
        issue_cols(0, 256 + min(2, K_PACK) * GP)
        for j in range(2, K_PACK, 2):
            issue_cols(xoff(j), xoff(min(j + 2, K_PACK) - 1) + GP)
        for g in range(K_PACK, GPC, 2):
            b = min(g + 2, GPC)
            issue_cols(xoff(g), xoff(b - 1) + G)
        for c in range(0, NAT, 2):
            n = min(2, NAT - c)
            q = min(qload, key=lambda k: qload[k])
            qload[q] += n * T * F * 0.3855
            qeng[q].dma_start(
                xn_all[:, c * T * F : (c + n) * T * F],
                XN[:, c * T * F : (c + n) * T * F],
            )

        # ---- u matmuls; transposes for packed graphs; exp per batch ----
        def u_graph(g, u_ps, base):
            goff = xoff(g)
            if g < K_PACK:
                for q in range(QP):
                    v = xall[:, goff + q * 256 : goff + (q + 1) * 256].rearrange(
                        "f (n two) -> f two n", two=2
                    )
                    for par in range(2):
                        c0 = base + (q * 2 + par) * H
                        nc.tensor.matmul(
                            u_ps[:, c0 : c0 + H],
                            v[:, par, :],
                            qkt[:, g * H : (g + 1) * H],
                            start=True,
                            stop=True,
                        )
            else:
                for t in range(T):
                    nc.tensor.matmul(
                        u_ps[:, base + t * H : base + (t + 1) * H],
                        xall[:, goff + t * 128 : goff + (t + 1) * 128],
                        qkt[:, g * H : (g + 1) * H],
                        start=True,
                        stop=True,
                    )

        # transposes + pair evacuation, interleaved with u in PE order so the
        # DVE evac chain starts as soon as each packed pair's slab lands.
        def transpose_pair(j):
            g0 = 2 * j
            npair = min(2, K_PACK - g0)
            tp = ps_t.tile([128, npair * QP * 128], fp16, tag="tp", name=f"tp{j}")
            for s in range(npair):
                sl = xall[:, xoff(g0 + s) : xoff(g0 + s) + GP].bitcast(fp16)
                for q in range(QP):
                    nc.tensor.transpose(
                        tp[:, (s * QP + q) * 128 : (s * QP + q + 1) * 128],
                        sl[:, q * 128 : (q + 1) * 128],
                        ident[:],
                    )
            nc.vector.tensor_copy(xnp[j][:, 0 : npair * QP * 128], tp[:])

        done_u = 0
        for grp in groups:
            w = sum(PH if g < K_PACK else TH for g in grp)
            u_ps = ps_u.tile([128, w], f32, tag="u", name=f"u{grp[0]}")
            base = 0
            for g in grp:
                u_graph(g, u_ps, base)
                base += PH if g < K_PACK else TH
                if g % 2 == 1 and g < K_PACK:
                    transpose_pair(g // 2)
            nc.scalar.activation(
                e_all[:, e_off[grp[0]] : e_off[grp[0]] + w], u_ps[:], AFT.Exp
            )

        # ---- S matmuls (need only e) ----
        s_a = ps_o.tile([PH, max(K_PACK, 1)], f32)
        s_b = ps_o.tile([TH, max(NAT, 1)], f32)
        for g in range(GPC):
            if g < K_PACK:
                nc.tensor.matmul(
                    s_a[:, g : g + 1],
                    e_all[:, e_off[g] : e_off[g] + PH],
                    ones[:],
                    start=True,
                    stop=True,
                )
            else:
                nc.tensor.matmul(
                    s_b[:, g - K_PACK : g - K_PACK + 1],
                    e_all[:, e_off[g] : e_off[g] + TH],
                    ones[:],
                    start=True,
                    stop=True,
                )

        # ---- xe matmuls: packed first, natural last; last graph separate ----
        xe_a = ps_o.tile([128, (GPC - 1) * H], f32)
        xe_b = ps_o.tile([128, H], f32)
        for g in range(GPC):
            dst = xe_b[:] if g == GPC - 1 else xe_a[:, g * H : (g + 1) * H]
            if g < K_PACK:
                j, s = g // 2, g % 2
                for q in range(QP):
                    v = (
                        xnp[j][:, (s * QP + q) * 128 : (s * QP + q + 1) * 128]
                        .bitcast(fp8)
                        .rearrange("p (f two) -> p two f", two=2)
                    )
                    for par in range(2):
                        nc.tensor.matmul(
                            dst,
                            v[:, par, :],
                            e_all[
                                :,
                                e_off[g] + (q * 2 + par) * H : e_off[g]
                                + (q * 2 + par + 1) * H,
                            ],
                            start=(q == 0 and par == 0),
                            stop=(q == QP - 1 and par == 1),
                        )
            else:
                i = g - K_PACK
                for t in range(T):
                    nc.tensor.matmul(
                        dst,
                        xn_all[:, i * T * F + t * F : i * T * F + (t + 1) * F],
                        e_all[:, e_off[g] + t * H : e_off[g] + (t + 1) * H],
                        start=(t == 0),
                        stop=(t == T - 1),
                    )
            if g == GPC - 2:
                nc.vector.tensor_copy(
                    out_sb[0:PH, GPC * H : GPC * H + K_PACK], s_a[:]
                )
                nc.vector.tensor_copy(out_sb[0:TH, GPC * H + K_PACK :], s_b[:])
                nc.vector.tensor_copy(out_sb[:, 0 : (GPC - 1) * H], xe_a[:])
        nc.vector.tensor_copy(out_sb[:, (GPC - 1) * H : GPC * H], xe_b[:])
        nc.sync.dma_start(OUT[:], out_sb[:])

    nc.compile()
    return nc


def _get(G):
    if G not in _CACHE:
        _CACHE[G] = _build(G)
    return _CACHE[G]


def _prepare(x, batch, context, Wq, Wk):
    """Host-side shard prep. Returns (in_maps, G, query, n_pad)."""
    counts = np.bincount(batch, minlength=B).astype(np.int64)
    G = int(np.ceil(max(int(counts.max()), 1) / 128.0) * 128)
    T = G // 128
    GP = ((G + 255) // 256) * 256
    starts = np.zeros(B + 1, np.int64)
    np.cumsum(counts, out=starts[1:])

    query = np.einsum("bc,hcv->hbv", context, Wq).astype(np.float32)  # [H,B,Dv]
    qk = np.einsum("hbv,hev->hbe", query, Wk).astype(np.float32)      # [H,B,F]
    qk8 = (qk / 8.0).astype(np.float16)

    x8 = x.astype(FP8)
    Wall = 256 + K_PACK * GP + NAT * G

    n_pad = np.zeros(B, np.float32)
    in_maps = []
    for c in range(N_CORES):
        XALLc = np.zeros((F, Wall), FP8)
        XNc = np.zeros((128, max(NAT, 1) * T * F), FP8)
        QKTc = np.zeros((F, GPC * H), np.float16)
        for gi in range(GPC):
            b = c * GPC + gi
            n0, n1 = int(starts[b]), int(starts[b + 1])
            cnt = n1 - n0
            if gi < K_PACK:
                o = 256 + gi * GP
                XALLc[:, o : o + cnt] = x8[n0:n1].T
                n_pad[b] = GP - cnt
            else:
                o = 256 + K_PACK * GP + (gi - K_PACK) * G
                XALLc[:, o : o + cnt] = x8[n0:n1].T
                buf = np.zeros((T * 128, F), FP8)
                buf[0:cnt] = x8[n0:n1]
                i = gi - K_PACK
                XNc[:, i * T * F : (i + 1) * T * F] = (
                    buf.reshape(T, 128, F).transpose(1, 0, 2).reshape(128, T * F)
                )
                n_pad[b] = G - cnt
            QKTc[:, gi * H : (gi + 1) * H] = qk8[:, b, :].T
        XALLc[:, 0:256] = QKTc.view(FP8)
        in_maps.append({"XALL": XALLc, "XN": XNc})
    return in_maps, G, query, n_pad


def kernel(**inputs):
    x = np.asarray(inputs["x"], np.float32)
    batch = np.asarray(inputs["batch"]).astype(np.int64)
    context = np.asarray(inputs["context"], np.float32)
    Wq = np.asarray(inputs["Wq"], np.float32)
    Wk = np.asarray(inputs["Wk"], np.float32)
    Wv = np.asarray(inputs["Wv"], np.float32)
    qc = float(np.asarray(inputs["query_coef"]).reshape(-1)[0])
    Wf = np.asarray(inputs["Wf"], np.float32)

    in_maps, G, query, n_pad = _prepare(x, batch, context, Wq, Wk)
    T = G // 128
    GP = ((G + 255) // 256) * 256
    QP = GP // 256

    nc = _get(G)
    res = run_bass_kernel_spmd(nc, in_maps, core_ids=list(range(N_CORES)))

    XE = np.zeros((H, B, F), np.float32)
    S = np.zeros((H, B), np.float32)
    for c in range(N_CORES):
        out = res.results[c]["OUT"]                      # [128, GPC*H + GPC]
        xeT = out[:, 0 : GPC * H].reshape(F, GPC, H)     # [f, g, h]
        XE[:, c * GPC : (c + 1) * GPC, :] = xeT.transpose(2, 1, 0)
        s = out[:, GPC * H :]
        if K_PACK > 0:
            sa = s[0 : QP * 2 * H, 0:K_PACK].reshape(QP * 2, H, K_PACK)
            S[:, c * GPC : c * GPC + K_PACK] = sa.sum(axis=0)
        if NAT > 0:
            sb = s[0 : T * H, K_PACK:GPC].reshape(T, H, NAT)
            S[:, c * GPC + K_PACK : (c + 1) * GPC] = sb.sum(axis=0)

    S = S - n_pad[None, :]  # pad slots contributed exp(0)*1 each to S
    Y = np.einsum("hbe,hev->hbv", XE, Wv.astype(np.float32))
    agg = Y / (S[..., None] + 1e-16)
    hbv = qc * query + agg
    out = np.einsum("hbv,ve->be", hbv, Wf)
    return out.astype(np.float32)


# revision 11
# speedup vs baseline: 1.1858x; 1.0065x over previous
"""Trainium2 Bass kernel for nn_AttentionLayerDecoder (sparse segment attention).

Math (reference, edge_index unused):
  qk[h,b,:]   = Wk[h] @ (context[b] @ Wq[h]) / 8          (tiny, host)
  u[h,n]      = x[n,:] . qk[h,batch[n],:]
  e[h,n]      = exp(u[h,n])                                (u ~ N(0,1))
  xe[h,b,:]   = sum_{n in b} e[h,n] * x[n,:]               (device)
  S[h,b]      = sum_{n in b} e[h,n]                        (device)
  out         = (qc*query + (xe @ Wv)/S) @ Wf, summed over heads  (tiny, host)

Device design (CoreSim cost-model driven):
  - A matmul is charged only for its OUTPUT free size (weight load is free),
    so both big contractions keep x as the *stationary* operand:
      u:  out[node,h]  = lhsT(x^T [f,node]) ^T @ qk[f,h]     -> 8 rows/tile
      xe: out[f,h]    += lhsT(x   [node,f]) ^T @ e[node,h]   -> 8 rows/tile
  - DMA transfer bytes are charged to the issuing queue; SP / Activation
    (HWDGE) / Pool (SWDGE) run in parallel. x ships in fp8 (rel err 3e-3 <<
    2e-2 budget) from one flat f-major tensor XALL = [qkt | packed | natural],
    sliced into ~2-graph chunks spread greedily over the three queues.
  - x is needed in BOTH layouts. "Natural" graphs get a second fp8 DMA copy
    (node-major, tensor XN). "Packed" graphs (processed first) instead build
    it on-chip: an fp16-viewed PE transpose moves TWO fp8 nodes per PE row,
    DVE evacuates two graphs per copy, and the xe matmul reads a stride-2
    fp8 view with even/odd node parities as two accumulating matmuls.
  - exp on ACT in two ~[128,480] batches; S via ones-matmuls; the xe PSUM is
    split so the last graph's matmuls never wait on the bulk evacuation.
"""

import sys

if "/opt/trn_rl_repo" not in sys.path:
    sys.path.insert(0, "/opt/trn_rl_repo")

from contextlib import ExitStack

import ml_dtypes
import numpy as np

import concourse.bass as bass
import concourse.tile as tile
from concourse import bacc, masks, mybir
from concourse.bass_utils import run_bass_kernel_spmd

N_CORES = 8
H = 8          # heads
F = 128        # node feature dim
B = 128        # graphs
GPC = B // N_CORES  # graphs per core

K_PACK = 6             # graphs whose node-major copy is built by PE transpose
NAT = GPC - K_PACK     # graphs that get a second (node-major) DMA copy

FP8 = ml_dtypes.float8_e4m3

_CACHE = {}


def _build(G):
    T = G // 128
    GP = ((G + 255) // 256) * 256   # packed graphs padded to 256-multiples
    QP = GP // 256                  # 256-node groups per packed graph
    TH = T * H                      # e columns per natural graph
    PH = QP * 2 * H                 # e columns per packed graph

    nc = bacc.Bacc(None, target_bir_lowering=False)
    fp8 = mybir.dt.float8e4
    fp16 = mybir.dt.float16
    f32 = mybir.dt.float32
    AFT = mybir.ActivationFunctionType

    W = 256 + K_PACK * GP + NAT * G   # qkt | packed slabs | natural slabs
    XALL = nc.dram_tensor("XALL", [F, W], fp8, kind="ExternalInput")
    XN = nc.dram_tensor("XN", [128, max(NAT, 1) * T * F], fp8, kind="ExternalInput")
    OUT = nc.dram_tensor("OUT", [128, GPC * H + GPC], f32, kind="ExternalOutput")

    def xoff(g):          # column offset of graph g's f-major slab in XALL
        if g < K_PACK:
            return 256 + g * GP
        return 256 + K_PACK * GP + (g - K_PACK) * G

    # e-column offsets: packed graphs (width PH) then natural (width TH)
    e_off, off = [], 0
    for g in range(GPC):
        e_off.append(off)
        off += PH if g < K_PACK else TH
    e_tot = off
    # exp batches (consecutive graphs, PSUM bank limit 512 f32 columns)
    groups, cur, w = [], [], 0
    for g in range(GPC):
        gw = PH if g < K_PACK else TH
        if w + gw > 512:
            groups.append(cur)
            cur, w = [], 0
        cur.append(g)
        w += gw
    groups.append(cur)

    with tile.TileContext(nc) as tc, ExitStack() as ctx:
        const = ctx.enter_context(tc.tile_pool(name="const", bufs=1))
        xpool = ctx.enter_context(tc.tile_pool(name="x", bufs=1))
        epool = ctx.enter_context(tc.tile_pool(name="e", bufs=1))
        outp = ctx.enter_context(tc.tile_pool(name="outp", bufs=1))
        ps_u = ctx.enter_context(
            tc.tile_pool(name="ps_u", bufs=2, space=bass.MemorySpace.PSUM)
        )
        ps_t = ctx.enter_context(
            tc.tile_pool(name="ps_t", bufs=3, space=bass.MemorySpace.PSUM)
        )
        ps_o = ctx.enter_context(
            tc.tile_pool(name="ps_o", bufs=1, space=bass.MemorySpace.PSUM)
        )

        ones = const.tile([128, 1], fp16)
        warm = const.tile([1, 1], f32)
        warm2 = const.tile([1, 1], fp16)
        ident = const.tile([128, 128], fp16)
        out_sb = outp.tile([128, GPC * H + GPC], f32)
        nc.gpsimd.memset(ones[:], 1.0)
        nc.gpsimd.memset(warm[:], 0.0)
        masks.make_identity(nc, ident[:])
        # Warm the ACT exp table at t~0 (1283ns table load hides under DMA).
        nc.scalar.activation(warm2[:], warm[:], AFT.Exp)

        xall = xpool.tile([128, W], fp8)
        xn_all = xpool.tile([128, max(NAT, 1) * T * F], fp8)
        # one SBUF tile per packed PAIR: [g, g+1] transposed, fp16-packed
        xnp = [
            xpool.tile([128, 2 * QP * 128], fp16, name=f"xnp{j}")
            for j in range((K_PACK + 1) // 2)
        ]
        e_all = epool.tile([128, e_tot], fp16)
        qkt = xall[:, 0:256].bitcast(fp16)   # [128, GPC*H]

        # ---- DMA schedule: greedy by projected queue completion ----
        qload = {"sp": 500.0, "pool": 650.0, "act": 1283.0 + 1200.0}
        qeng = {"sp": nc.sync, "pool": nc.gpsimd, "act": nc.scalar}

        def issue_cols(a, b):
            q = min(qload, key=lambda k: qload[k])
            qload[q] += (b - a) * 0.3855
            qeng[q].dma_start(xall[:, a:b], XALL[:, a:b])

        # first chunk carries qkt + packed pair 0; then packed pairs, natural
        # pairs, and finally the XN (node-major) chunks that feed the tail.
        issue_cols(0, 256 + min(2, K_PACK) * GP)
        for j in range(2, K_PACK, 2):
            issue_cols(xoff(j), xoff(min(j + 2, K_PACK) - 1) + GP)
        for g in range(K_PACK, GPC, 2):
            b = min(g + 2, GPC)
            issue_cols(xoff(g), xoff(b - 1) + G)
        for c in range(0, NAT, 2):
            n = min(2, NAT - c)
            q = min(qload, key=lambda k: qload[k])
            qload[q] += n * T * F * 0.3855
            qeng[q].dma_start(
                xn_all[:, c * T * F : (c + n) * T * F],
                XN[:, c * T * F : (c + n) * T * F],
            )

        # ---- u matmuls; transposes for packed graphs; exp per batch ----
        def u_graph(g, u_ps, base):
            goff = xoff(g)
            if g < K_PACK:
                for q in range(QP):
                    v = xall[:, goff + q * 256 : goff + (q + 1) * 256].rearrange(
                        "f (n two) -> f two n", two=2
                    )
                    for par in range(2):
                        c0 = base + (q * 2 + par) * H
                        nc.tensor.matmul(
                            u_ps[:, c0 : c0 + H],
                            v[:, par, :],
                            qkt[:, g * H : (g + 1) * H],
                            start=True,
                            stop=True,
                        )
            else:
                for t in range(T):
                    nc.tensor.matmul(
                        u_ps[:, base + t * H : base + (t + 1) * H],
                        xall[:, goff + t * 128 : goff + (t + 1) * 128],
                        qkt[:, g * H : (g + 1) * H],
                        start=True,
                        stop=True,
                    )

        # transposes + pair evacuation, interleaved with u in PE order so the
        # DVE evac chain starts as soon as each packed pair's slab lands.
        def transpose_pair(j):
            g0 = 2 * j
            npair = min(2, K_PACK - g0)
            tp = ps_t.tile([128, npair * QP * 128], fp16, tag="tp", name=f"tp{j}")
            for s in range(npair):
                sl = xall[:, xoff(g0 + s) : xoff(g0 + s) + GP].bitcast(fp16)
                for q in range(QP):
                    nc.tensor.transpose(
                        tp[:, (s * QP + q) * 128 : (s * QP + q + 1) * 128],
                        sl[:, q * 128 : (q + 1) * 128],
                        ident[:],
                    )
            nc.vector.tensor_copy(xnp[j][:, 0 : npair * QP * 128], tp[:])

        done_u = 0
        for grp in groups:
            w = sum(PH if g < K_PACK else TH for g in grp)
            u_ps = ps_u.tile([128, w], f32, tag="u", name=f"u{grp[0]}")
            base = 0
            for g in grp:
                u_graph(g, u_ps, base)
                base += PH if g < K_PACK else TH
                if g % 2 == 1 and g < K_PACK:
                    transpose_pair(g // 2)
            nc.scalar.activation(
                e_all[:, e_off[grp[0]] : e_off[grp[0]] + w], u_ps[:], AFT.Exp
            )

        # ---- S matmuls (need only e); S + last graph's xe share a tile ----
        # combo layout: cols 0:H = xe of the last graph, cols H:H+GPC = S.
        xe_a = ps_o.tile([128, (GPC - 1) * H], f32)
        combo = ps_o.tile([128, H + GPC], f32)
        nc.vector.memset(combo[:], 0.0)
        for g in range(GPC):
            w = PH if g < K_PACK else TH
            nc.tensor.matmul(
                combo[0:w, H + g : H + g + 1],
                e_all[:, e_off[g] : e_off[g] + w],
                ones[:],
                start=True,
                stop=True,
            )
        for g in range(GPC):
            dst = (
                combo[:, 0:H] if g == GPC - 1 else xe_a[:, g * H : (g + 1) * H]
            )
            if g < K_PACK:
                j, s = g // 2, g % 2
                for q in range(QP):
                    v = (
                        xnp[j][:, (s * QP + q) * 128 : (s * QP + q + 1) * 128]
                        .bitcast(fp8)
                        .rearrange("p (f two) -> p two f", two=2)
                    )
                    for par in range(2):
                        nc.tensor.matmul(
                            dst,
                            v[:, par, :],
                            e_all[
                                :,
                                e_off[g] + (q * 2 + par) * H : e_off[g]
                                + (q * 2 + par + 1) * H,
                            ],
                            start=(q == 0 and par == 0),
                            stop=(q == QP - 1 and par == 1),
                        )
            else:
                i = g - K_PACK
                for t in range(T):
                    nc.tensor.matmul(
                        dst,
                        xn_all[:, i * T * F + t * F : i * T * F + (t + 1) * F],
                        e_all[:, e_off[g] + t * H : e_off[g] + (t + 1) * H],
                        start=(t == 0),
                        stop=(t == T - 1),
                    )
            if g == GPC - 2:
                nc.vector.tensor_copy(out_sb[:, 0 : (GPC - 1) * H], xe_a[:])
        # combo holds [xe(last graph) | S for all graphs]
        nc.vector.tensor_copy(out_sb[:, (GPC - 1) * H :], combo[:])
        nc.sync.dma_start(OUT[:], out_sb[:])

    nc.compile()
    return nc


def _get(G):
    if G not in _CACHE:
        _CACHE[G] = _build(G)
    return _CACHE[G]


def _prepare(x, batch, context, Wq, Wk):
    """Host-side shard prep. Returns (in_maps, G, query, n_pad)."""
    counts = np.bincount(batch, minlength=B).astype(np.int64)
    G = int(np.ceil(max(int(counts.max()), 1) / 128.0) * 128)
    T = G // 128
    GP = ((G + 255) // 256) * 256
    starts = np.zeros(B + 1, np.int64)
    np.cumsum(counts, out=starts[1:])

    query = np.einsum("bc,hcv->hbv", context, Wq).astype(np.float32)  # [H,B,Dv]
    qk = np.einsum("hbv,hev->hbe", query, Wk).astype(np.float32)      # [H,B,F]
    qk8 = (qk / 8.0).astype(np.float16)

    x8 = x.astype(FP8)
    Wall = 256 + K_PACK * GP + NAT * G

    n_pad = np.zeros(B, np.float32)
    in_maps = []
    for c in range(N_CORES):
        XALLc = np.zeros((F, Wall), FP8)
        XNc = np.zeros((128, max(NAT, 1) * T * F), FP8)
        QKTc = np.zeros((F, GPC * H), np.float16)
        for gi in range(GPC):
            b = c * GPC + gi
            n0, n1 = int(starts[b]), int(starts[b + 1])
            cnt = n1 - n0
            if gi < K_PACK:
                o = 256 + gi * GP
                XALLc[:, o : o + cnt] = x8[n0:n1].T
                n_pad[b] = GP - cnt
            else:
                o = 256 + K_PACK * GP + (gi - K_PACK) * G
                XALLc[:, o : o + cnt] = x8[n0:n1].T
                buf = np.zeros((T * 128, F), FP8)
                buf[0:cnt] = x8[n0:n1]
                i = gi - K_PACK
                XNc[:, i * T * F : (i + 1) * T * F] = (
                    buf.reshape(T, 128, F).transpose(1, 0, 2).reshape(128, T * F)
                )
                n_pad[b] = G - cnt
            QKTc[:, gi * H : (gi + 1) * H] = qk8[:, b, :].T
        XALLc[:, 0:256] = QKTc.view(FP8)
        in_maps.append({"XALL": XALLc, "XN": XNc})
    return in_maps, G, query, n_pad


def kernel(**inputs):
    x = np.asarray(inputs["x"], np.float32)
    batch = np.asarray(inputs["batch"]).astype(np.int64)
    context = np.asarray(inputs["context"], np.float32)
    Wq = np.asarray(inputs["Wq"], np.float32)
    Wk = np.asarray(inputs["Wk"], np.float32)
    Wv = np.asarray(inputs["Wv"], np.float32)
    qc = float(np.asarray(inputs["query_coef"]).reshape(-1)[0])
    Wf = np.asarray(inputs["Wf"], np.float32)

    in_maps, G, query, n_pad = _prepare(x, batch, context, Wq, Wk)
    T = G // 128
    GP = ((G + 255) // 256) * 256
    QP = GP // 256

    nc = _get(G)
    res = run_bass_kernel_spmd(nc, in_maps, core_ids=list(range(N_CORES)))

    XE = np.zeros((H, B, F), np.float32)
    S = np.zeros((H, B), np.float32)
    for c in range(N_CORES):
        out = res.results[c]["OUT"]                      # [128, GPC*H + GPC]
        xeT = out[:, 0 : GPC * H].reshape(F, GPC, H)     # [f, g, h]
        XE[:, c * GPC : (c + 1) * GPC, :] = xeT.transpose(2, 1, 0)
        s = out[:, GPC * H :]
        if K_PACK > 0:
            sa = s[0 : QP * 2 * H, 0:K_PACK].reshape(QP * 2, H, K_PACK)
            S[:, c * GPC : c * GPC + K_PACK] = sa.sum(axis=0)
        if NAT > 0:
            sb = s[0 : T * H, K_PACK:GPC].reshape(T, H, NAT)
            S[:, c * GPC + K_PACK : (c + 1) * GPC] = sb.sum(axis=0)

    S = S - n_pad[None, :]  # pad slots contributed exp(0)*1 each to S
    Y = np.einsum("hbe,hev->hbv", XE, Wv.astype(np.float32))
    agg = Y / (S[..., None] + 1e-16)
    hbv = qc * query + agg
    out = np.einsum("hbv,ve->be", hbv, Wf)
    return out.astype(np.float32)
